# revision 1
# baseline (speedup 1.0000x reference)
"""KGCN aggregate kernel for 8 Trainium2 NeuronCores.

Strategy: nodes are sharded across the 8 cores (12500 each). On the host,
edges are sorted by destination and bucketed into per-core 128-node windows.
On each core, for every window, edge tiles of 128 are processed as:
  gather 128 source rows (indirect DMA) -> weighted one-hot (DVE, one
  tensor_scalar) -> PE matmul accumulating the segment sum in PSUM.
The window epilogue adds emb_dst, applies the linear layer via a PE
transpose + matmul, adds bias, applies tanh and stores the output rows.
No cross-core collective is needed; each core owns its node range.
"""
import numpy as np

N = 100000
E = 1600000
D = 64
NCORES = 8
NPC = N // NCORES          # nodes per core
P = 128
WPC = (NPC + P - 1) // P   # windows per core (98; last window is partial)
NPAD = WPC * P             # padded nodes per core (12544)

_TRACE = False             # set by test harness to collect exec time
LAST_EXEC_NS = None


def _prep(emb_dst, edge_weight, src_idx, dst_idx):
    """Sort edges by dst, bucket into (core, window) cells, pad each cell
    to whole 128-slot tiles (uniform tile count per window across cores)."""
    src = np.asarray(src_idx).astype(np.int32)
    dst = np.asarray(dst_idx).astype(np.int64)
    w = np.asarray(edge_weight).astype(np.float32)

    order = np.argsort(dst, kind="stable")
    ds = dst[order]
    ss = src[order]
    ws = w[order]

    core = ds // NPC
    rel = ds - core * NPC
    gw = core * WPC + rel // P
    cnt = np.bincount(gw, minlength=NCORES * WPC).reshape(NCORES, WPC)
    tw = np.maximum(1, -(-cnt // P)).max(axis=0).astype(np.int64)  # [WPC]
    tile_base = np.zeros(WPC + 1, np.int64)
    tile_base[1:] = np.cumsum(tw)
    TT = int(tile_base[-1])        # tiles per core
    S = TT * P

    idx_slots = np.zeros((NCORES, S), np.int32)
    w_slots = np.zeros((NCORES, S), np.float32)
    dr_slots = np.zeros((NCORES, S), np.float32)

    node_marks = (
        (np.arange(NCORES * WPC) // WPC) * NPC
        + (np.arange(NCORES * WPC) % WPC) * P
    )
    starts = np.searchsorted(ds, node_marks)
    ends = np.append(starts[1:], E)
    for c in range(NCORES):
        for wi in range(WPC):
            k = c * WPC + wi
            a, bnd = starts[k], ends[k]
            n = bnd - a
            if n == 0:
                continue
            off = int(tile_base[wi]) * P
            idx_slots[c, off : off + n] = ss[a:bnd]
            w_slots[c, off : off + n] = ws[a:bnd]
            dr_slots[c, off : off + n] = (
                ds[a:bnd] - (c * NPC + wi * P)
            ).astype(np.float32)

    def wrap(x):  # slot i -> (partition i%128, tile i//128)
        return np.ascontiguousarray(x.reshape(NCORES, TT, P).transpose(0, 2, 1))

    return wrap(idx_slots), wrap(w_slots), wrap(dr_slots), tw, tile_base, TT


def _build(tw, tile_base, TT):
    import concourse.bacc as bacc
    import concourse.bass as bass
    import concourse.mybir as mybir
    import concourse.tile as tile
    from concourse.masks import make_identity

    dt = mybir.dt
    nc = bacc.Bacc("TRN2", target_bir_lowering=False, debug=False)
    t_embsrc = nc.dram_tensor("embsrc", [N, D], dt.float32, kind="ExternalInput")
    t_embdst = nc.dram_tensor("embdst", [NPAD, D], dt.float32, kind="ExternalInput")
    t_idx = nc.dram_tensor("idx", [P, TT], dt.int32, kind="ExternalInput")
    t_w = nc.dram_tensor("w", [P, TT], dt.float32, kind="ExternalInput")
    t_dr = nc.dram_tensor("dr", [P, TT], dt.float32, kind="ExternalInput")
    t_W = nc.dram_tensor("lin_w", [D, D], dt.float32, kind="ExternalInput")
    t_bb = nc.dram_tensor("b_bcast", [P, D], dt.float32, kind="ExternalInput")
    t_iota = nc.dram_tensor("iota", [P, P], dt.float32, kind="ExternalInput")
    t_out = nc.dram_tensor("out", [NPAD, D], dt.float32, kind="ExternalOutput")

    with tile.TileContext(nc) as tc:
        with (
            tc.tile_pool(name="const", bufs=1) as constp,
            tc.tile_pool(name="gp", bufs=12) as gp,
            tc.tile_pool(name="ohp", bufs=8) as ohp,
            tc.tile_pool(name="winp", bufs=3) as winp,
            tc.tile_pool(name="psA", bufs=2, space="PSUM") as psA,
            tc.tile_pool(name="psB", bufs=2, space="PSUM") as psB,
        ):
            idx_sb = constp.tile([P, TT], dt.int32)
            nc.sync.dma_start(out=idx_sb[:], in_=t_idx[:])
            w_sb = constp.tile([P, TT], dt.float32)
            nc.sync.dma_start(out=w_sb[:], in_=t_w[:])
            dr_sb = constp.tile([P, TT], dt.float32)
            nc.sync.dma_start(out=dr_sb[:], in_=t_dr[:])
            iota_sb = constp.tile([P, P], dt.float32)
            nc.sync.dma_start(out=iota_sb[:], in_=t_iota[:])
            Wl_sb = constp.tile([D, D], dt.float32)
            nc.sync.dma_start(out=Wl_sb[:], in_=t_W[:])
            bb_sb = constp.tile([P, D], dt.float32)
            nc.sync.dma_start(out=bb_sb[:], in_=t_bb[:])
            ident = constp.tile([P, P], dt.float32)
            make_identity(nc, ident[:])

            for wi in range(WPC):
                nt = int(tw[wi])
                pf = psA.tile([P, D], dt.float32, tag="pf")
                for j in range(nt):
                    t = int(tile_base[wi]) + j
                    g = gp.tile([P, D], dt.float32, tag="g")
                    nc.gpsimd.indirect_dma_start(
                        out=g[:],
                        out_offset=None,
                        in_=t_embsrc[:],
                        in_offset=bass.IndirectOffsetOnAxis(
                            ap=idx_sb[:, t : t + 1], axis=0
                        ),
                    )
                    oh = ohp.tile([P, P], dt.float32, tag="oh")
                    nc.vector.tensor_scalar(
                        out=oh[:],
                        in0=iota_sb[:],
                        scalar1=dr_sb[:, t : t + 1],
                        scalar2=w_sb[:, t : t + 1],
                        op0=mybir.AluOpType.is_equal,
                        op1=mybir.AluOpType.mult,
                    )
                    nc.tensor.matmul(
                        out=pf[:],
                        lhsT=oh[:],
                        rhs=g[:],
                        start=(j == 0),
                        stop=(j == nt - 1),
                    )
                ed = winp.tile([P, D], dt.float32, tag="ed")
                nc.sync.dma_start(out=ed[:], in_=t_embdst[wi * P : (wi + 1) * P, :])
                h = winp.tile([P, D], dt.float32, tag="h")
                nc.vector.tensor_add(out=h[:], in0=pf[:], in1=ed[:])
                pT = psB.tile([D, P], dt.float32, tag="pT")
                nc.tensor.transpose(out=pT[:], in_=h[:], identity=ident[:])
                hT = winp.tile([D, P], dt.float32, tag="hT")
                nc.scalar.copy(out=hT[:], in_=pT[:])
                pl = psB.tile([P, D], dt.float32, tag="pl")
                nc.tensor.matmul(
                    out=pl[:], lhsT=hT[:], rhs=Wl_sb[:], start=True, stop=True
                )
                o = winp.tile([P, D], dt.float32, tag="o")
                nc.vector.tensor_add(out=o[:], in0=pl[:], in1=bb_sb[:])
                nc.scalar.activation(
                    out=o[:], in_=o[:], func=mybir.ActivationFunctionType.Tanh
                )
                nc.sync.dma_start(out=t_out[wi * P : (wi + 1) * P, :], in_=o[:])
    nc.compile()
    return nc


def kernel(emb_src, emb_dst, edge_weight, W, b, src_idx, dst_idx,
           user_indices, labels, perm):
    global LAST_EXEC_NS
    from concourse.bass_utils import run_bass_kernel_spmd

    emb_src = np.ascontiguousarray(np.asarray(emb_src, np.float32))
    emb_dst_np = np.ascontiguousarray(np.asarray(emb_dst, np.float32))
    W_np = np.ascontiguousarray(np.asarray(W, np.float32))
    b_np = np.asarray(b, np.float32)

    idx_w, w_w, dr_w, tw, tile_base, TT = _prep(
        emb_dst_np, edge_weight, src_idx, dst_idx
    )

    embdst_pad = np.zeros((NCORES, NPAD, D), np.float32)
    embdst_pad[:, :NPC] = emb_dst_np.reshape(NCORES, NPC, D)
    iota = np.ascontiguousarray(
        np.tile(np.arange(P, dtype=np.float32), (P, 1))
    )
    b_bcast = np.ascontiguousarray(np.tile(b_np, (P, 1)))

    nc = _build(tw, tile_base, TT)

    in_maps = []
    for c in range(NCORES):
        in_maps.append({
            "embsrc": emb_src,
            "embdst": embdst_pad[c],
            "idx": idx_w[c],
            "w": w_w[c],
            "dr": dr_w[c],
            "lin_w": W_np,
            "b_bcast": b_bcast,
            "iota": iota,
        })

    if _TRACE:
        try:
            import profhook
            profhook.install()
        except Exception:
            pass
    res = run_bass_kernel_spmd(nc, in_maps, list(range(NCORES)), trace=_TRACE)
    LAST_EXEC_NS = res.exec_time_ns

    out = np.concatenate(
        [res.results[c]["out"][:NPC] for c in range(NCORES)], axis=0
    )
    perm_np = np.asarray(perm)
    users = np.asarray(user_indices)[perm_np]
    labs = np.asarray(labels)[perm_np]
    return out, users, labs
